# revision 34
# baseline (speedup 1.0000x reference)
"""Multi-head attention (N=2, T=2048, D=1024, H=16, dk=dv=64) on 8 TRN2 cores.

Sharding: tensor-parallel over heads. Core p computes heads {2p, 2p+1}
(a 128-wide slice of the QKV projections and of WO's rows), producing a
partial output [2, 2048, 1024]; the host sums the 8 partials and adds bO
(row-parallel linear => sum-reduce unshard).

Device algorithm (per core, per batch n):
  1. qT = (WQp/8).T @ Q.T   [128, 2048]   (scale 1/sqrt(dk) folded into WQp)
     kT = WKp.T @ K.T       [128, 2048]
     v  = V @ WVp           [128part(l-tile), 16, 2, dk+1] with a ones
          column appended per head (softmax denominator for free)
  2. scores in "KQ" orientation: S^T[l, q] = kT.T(l-tile) @ qT(q-chunk);
     the two heads run CONCURRENTLY on PE row groups 0-63 / 64-127.
  3. E = exp(S^T) on ScalarE (its only job), PSUM -> SBUF bf16.
  4. attnT_aug[dv+1, q] += v_aug.T(l-tile) @ E accumulated over l-tiles in
     PSUM; row dv holds sum(exp) = softmax denominator.
  5. normalize: att = attnT * bcast(1/den) (DVE; 1/den broadcast across
     partitions via a K=1 PE matmul)
  6. O^T-partial: out[q-tile, :] = att[:, q-tile].T @ WOp, written fp16.

Schedule: a warm-up burst of dummy matmuls un-throttles the PE HAM clock
gate (4/8 -> 8/8) during the initial DMA wait; after that every piece of
projection / normalize / out-projection work is dripped into the
exp-paced attention l-loops so the PE stream never idles (no HAM
re-throttle) and ScalarE runs an uninterrupted exp stream.  All
PSUM->SBUF casts are on DVE, all DMA issue on the SP/GpSimd queues.
"""

import math
import numpy as np
from contextlib import ExitStack
from collections import deque

import concourse.bass as bass
import concourse.tile as tile
from concourse import bacc, mybir
from concourse.bass_utils import run_bass_kernel_spmd

N_CORES = 8
NB, T, D = 2, 2048, 1024
HEADS, DK = 16, 64
HP = 2 * DK          # per-core head-pair width = 128
QC = 512             # query-chunk (matmul moving free dim)
NQC = T // QC        # 4
LTS = 128            # key/l tile (PE partition dim)
NLT = T // LTS       # 16
CK = 128             # contraction chunk for projections
NCK = D // CK        # 8
VW = DK + 1          # v columns per head incl. ones column

F32 = mybir.dt.float32
BF16 = mybir.dt.bfloat16
FP16 = mybir.dt.float16
EXP = mybir.ActivationFunctionType.Exp


def build_program(mm_dt=BF16, out_dt=FP16):
    """Build + compile the SPMD program (identical on all 8 cores)."""
    nc = bacc.Bacc("TRN2", target_bir_lowering=False, debug=False,
                   num_devices=N_CORES)
    # [NB, NQC, CK, NCK*QC]: per (n, chunk) a contiguous [128, 8KB] block
    # so each input chunk is ONE dma_start with 2KB+ descriptors.
    QT = nc.dram_tensor("QT", [NB, NQC, CK, NCK, QC], mm_dt,
                        kind="ExternalInput").ap()
    KT = nc.dram_tensor("KT", [NB, NQC, CK, NCK, QC], mm_dt,
                        kind="ExternalInput").ap()
    VT = nc.dram_tensor("VT", [NB, NQC, CK, NCK, QC], mm_dt,
                        kind="ExternalInput").ap()
    WQp = nc.dram_tensor("WQp", [CK, NCK, HP], mm_dt,
                         kind="ExternalInput").ap()
    WKp = nc.dram_tensor("WKp", [CK, NCK, HP], mm_dt,
                         kind="ExternalInput").ap()
    WVp = nc.dram_tensor("WVp", [CK, NCK, HP], mm_dt,
                         kind="ExternalInput").ap()
    WOp = nc.dram_tensor("WOp", [HP, D], mm_dt, kind="ExternalInput").ap()
    O = nc.dram_tensor("O", [NB, T, D], out_dt, kind="ExternalOutput").ap()

    with tile.TileContext(nc) as tc, ExitStack() as ctx:
        wpool = ctx.enter_context(tc.tile_pool(name="w", bufs=1))
        seq = ctx.enter_context(tc.tile_pool(name="seq", bufs=2))
        inp = ctx.enter_context(tc.tile_pool(name="inp", bufs=11))
        epool = ctx.enter_context(tc.tile_pool(name="e", bufs=8))
        apool = ctx.enter_context(tc.tile_pool(name="att", bufs=3))
        opool = ctx.enter_context(tc.tile_pool(name="o", bufs=6))
        ppool = ctx.enter_context(tc.tile_pool(name="pp", bufs=2, space="PSUM"))
        spool = ctx.enter_context(tc.tile_pool(name="ps", bufs=2, space="PSUM"))
        atpool = ctx.enter_context(tc.tile_pool(name="pa", bufs=1, space="PSUM"))

        # --- static SBUF: weights + constants ---
        wq_s = wpool.tile([CK, NCK, HP], mm_dt)
        wk_s = wpool.tile([CK, NCK, HP], mm_dt)
        wv_s = wpool.tile([CK, NCK, HP], mm_dt)
        wo_s = wpool.tile([HP, D], mm_dt)

        ones_col = wpool.tile([1, DK], F32, name="ones_col")
        nc.vector.memset(ones_col, 1.0)
        ones_col_r = wpool.tile([1, DK], mm_dt, name="ones_col_r")
        nc.vector.tensor_copy(ones_col_r, ones_col)
        # warm-up operand (zeros are fine for dummy matmuls)
        wsrc = wpool.tile([CK, QC], mm_dt, name="wsrc")
        nc.vector.memset(wsrc, 0.125)

        # --- HAM warm-up: dummy matmuls that run during the initial DMA
        # wait. ~8 N=512 matmuls at the cold 1.2GHz clock cover the 3.4us
        # busy window that flips the PE clock gate to 8/8; a few more keep
        # it busy until the first real projection's inputs land. ---
        for wi in range(12):
            pw = ppool.tile([HP, QC], F32, tag="pp", name="warm")
            nc.tensor.matmul(pw, lhsT=wsrc[:, 0:HP], rhs=wsrc,
                             start=True, stop=True)

        # --- staged input chunks (one tile + one DMA per (src, n, c)) ---
        stage = {}

        def dma_issue8(src, n, c, key, queues):
            """Startup chunks: 8 per-ck sub-DMAs spread across queues
            (each issue-stream only sustains ~45GB/s, so landing latency
            scales with stream count), per-ck tiles so each projection
            matmul starts as soon as its own 128KB block lands."""
            for ck in range(NCK):
                cin = inp.tile([CK, QC], mm_dt, tag="cin", bufs=24,
                               name="cin")
                queues[ck % len(queues)].dma_start(
                    out=cin, in_=src[n, c, :, ck, :])
                stage[(key, ck)] = cin

        def dma_in(src, n, c, key, eng=None):
            """Prefetch chunks: 4 quarter-chunk sub-DMAs — a balance of
            issue cost (~650ns per dma_start, the scarce mid-run resource)
            and landing latency (~45GB/s per issue-stream)."""
            def f():
                q = eng or nc.sync
                for s in range(4):
                    cin = inp.tile([CK, 2, QC], mm_dt, tag="cin4", bufs=36,
                                   name="cin4")
                    q.dma_start(out=cin, in_=src[n, c, :, 2 * s:2 * s + 2, :])
                    stage[(key, 2 * s)] = cin[:, 0, :]
                    stage[(key, 2 * s + 1)] = cin[:, 1, :]
            return f

        def qk_mm(w_s, dst, qc, key):
            """Fused projection: 8 accumulating matmuls + one DVE cast."""
            def f():
                ps = ppool.tile([HP, QC], F32, tag="pp", name="ps_proj")
                for ck in range(NCK):
                    nc.tensor.matmul(ps, lhsT=w_s[:, ck, :],
                                     rhs=stage.pop((key, ck)),
                                     start=(ck == 0), stop=(ck == NCK - 1))
                nc.vector.tensor_copy(dst[:, qc * QC:(qc + 1) * QC], ps)
            return [f]

        def v_mm(v_sb, n, c, key):
            """v natural-layout projection for token-chunk c (4 l-tiles);
            one [128, 2, 64] DVE copy per l-tile (ones cols untouched)."""
            cins = {}
            def grab():
                for ck in range(NCK):
                    cins[ck] = stage.pop((key, ck))
            def t(j):
                def f():
                    lt = c * (QC // LTS) + j
                    pv = ppool.tile([LTS, 2, DK], F32, tag="pp", name="pv")
                    for ck in range(NCK):
                        nc.tensor.matmul(
                            pv, lhsT=cins[ck][:, j * LTS:(j + 1) * LTS],
                            rhs=wv_s[:, ck, :],
                            start=(ck == 0), stop=(ck == NCK - 1))
                    nc.vector.tensor_copy(v_sb[:, lt, :, 0:DK], pv)
                return f
            return [grab] + [t(j) for j in range(QC // LTS)]

        def scores_pair(qT_sb, kT_sb, qc, lt):
            """S^T for both heads of (q-chunk, l-tile) into one 2-bank PSUM
            tile; heads run concurrently on PE row groups; single wide exp."""
            ss = spool.tile([LTS, 2 * QC], F32, tag="ss", name="ss")
            for h in range(2):
                nc.tensor.matmul(
                    ss[:, h * QC:(h + 1) * QC],
                    lhsT=kT_sb[DK * h:DK * (h + 1), lt * LTS:(lt + 1) * LTS],
                    rhs=qT_sb[DK * h:DK * (h + 1), qc * QC:(qc + 1) * QC],
                    start=True, stop=True)
            e = epool.tile([LTS, 2 * QC], mm_dt, tag="e", name="e")
            nc.scalar.activation(e, ss, EXP)
            return e

        def av_pair(v_sb, ps_att, e, lt, start, stop):
            for h in range(2):
                nc.tensor.matmul(ps_att[h],
                                 lhsT=v_sb[:, lt, h, :],
                                 rhs=e[:, h * QC:(h + 1) * QC],
                                 start=start, stop=stop)

        def norm_thunks(ps_att):
            """Softmax normalization for a finished accumulator pair. The
            PSUM quick-release copies (tA) are emitted INLINE so the next
            chunk's first AV matmul (which reuses the single-buffered
            accumulator) never convoys behind later DVE work. Returns
            (att, [C, D], tDj): C builds the 1/den broadcast, D applies."""
            att_raw = apool.tile([HP, QC], F32, tag="att_raw", name="att_raw")
            att = apool.tile([HP, QC], mm_dt, tag="attT", name="att")
            state = {}

            def tA(h):
                def f():
                    nc.vector.tensor_copy(att_raw[DK * h:DK * (h + 1), :],
                                          ps_att[h][0:DK, :])
                    den_f = apool.tile([1, QC], F32, tag=f"den{h}",
                                       name="den_f")
                    nc.vector.tensor_copy(den_f, ps_att[h][DK:VW, :])
                    state["den%d" % h] = den_f
                return f

            def tC():
                for h in range(2):
                    den_rf = apool.tile([1, QC], F32, tag=f"denr{h}",
                                        name="den_rf")
                    nc.vector.reciprocal_approx_fast(den_rf,
                                                     state["den%d" % h])
                    den_rr = apool.tile([1, QC], mm_dt, tag=f"denrr{h}",
                                        name="den_rr")
                    nc.vector.tensor_copy(den_rr, den_rf)
                    bcp = ppool.tile([DK, QC], F32, tag="pp", name="bc_ps")
                    nc.tensor.matmul(bcp, lhsT=ones_col_r, rhs=den_rr,
                                     start=True, stop=True)
                    state["bc%d" % h] = bcp

            def tD():
                for h in range(2):
                    nc.vector.tensor_mul(att[DK * h:DK * (h + 1), :],
                                         att_raw[DK * h:DK * (h + 1), :],
                                         state["bc%d" % h])

            def tDj(j):
                sl = slice(j * LTS, (j + 1) * LTS)
                for h in range(2):
                    nc.vector.tensor_mul(att[DK * h:DK * (h + 1), sl],
                                         att_raw[DK * h:DK * (h + 1), sl],
                                         state["bc%d" % h][:, sl])

            tA(0)()
            tA(1)()
            return att, [tC, tD], tDj

        def out_proj_thunks(n, att, qc, queues=None):
            """out-projection chunk: 8 (MM + fp16-cast) thunks, DMA per
            q-tile issued from `queues` (round-robin)."""
            queues = queues or [nc.gpsimd]
            box = {}
            thunks = []
            for j in range(QC // LTS):
                for half in range(2):
                    def t(j=j, half=half):
                        qt = qc * (QC // LTS) + j
                        if half == 0:
                            box[j] = opool.tile([LTS, D], out_dt, tag="osb",
                                                name="o_sb")
                        o_sb = box[j]
                        po = ppool.tile([LTS, QC], F32, tag="pp", name="po")
                        nc.tensor.matmul(
                            po, lhsT=att[:, j * LTS:(j + 1) * LTS],
                            rhs=wo_s[:, half * QC:(half + 1) * QC],
                            start=True, stop=True)
                        nc.vector.tensor_copy(
                            o_sb[:, half * QC:(half + 1) * QC], po)
                        if half == 1:
                            queues[j % len(queues)].dma_start(
                                out=O[n, qt * LTS:(qt + 1) * LTS, :],
                                in_=box.pop(j))
                    thunks.append(t)
            return thunks

        def body():
            seqs = []
            for n in range(NB):
                qT_sb = seq.tile([HP, T], mm_dt, tag="qT", name="qT_sb")
                kT_sb = seq.tile([HP, T], mm_dt, tag="kT", name="kT_sb")
                v_sb = seq.tile([LTS, NLT, 2, VW], mm_dt, tag="v",
                                name="v_sb")
                nc.vector.memset(v_sb[:, :, :, DK], 1.0)
                seqs.append((qT_sb, kT_sb, v_sb))

            def kv_proj(n, c):
                """3 drip thunks: k-proj burst, then v j-tiles 0-1, 2-3."""
                k = qk_mm(wk_s, seqs[n][1], c, ("k", n, c))
                v = v_mm(seqs[n][2], n, c, ("v", n, c))
                return [k[0], lambda: [t() for t in v[0:3]],
                        lambda: [t() for t in v[3:5]]]

            def dmas(*specs):
                """One thunk per chunk DMA (4 issues each) on gpsimd."""
                return [dma_in(src, n, c, (pfx, n, c), eng=nc.gpsimd)
                        for (src, n, c, pfx) in specs]

            def dmas_sp(*specs):
                """Same, issued from the SP queue."""
                return [dma_in(src, n, c, (pfx, n, c))
                        for (src, n, c, pfx) in specs]

            def warm(k):
                for _ in range(k):
                    pw = ppool.tile([HP, QC], F32, tag="pp", name="warm")
                    nc.tensor.matmul(pw, lhsT=wsrc[:, 0:HP], rhs=wsrc,
                                     start=True, stop=True)

            def tail(ps_att, n, qc):
                """Final-chunk normalize + out-project, pipelined per
                q-tile with the norm chain split across DVE and the (now
                idle) ScalarE; warm matmuls keep the HAM clock at 8/8."""
                att = apool.tile([HP, QC], mm_dt, tag="attT", name="att")
                den, rcp, rr = {}, {}, {}
                den[0] = apool.tile([1, QC], F32, tag="den0", name="den_f")
                nc.vector.tensor_copy(den[0], ps_att[0][DK:VW, :])
                den[1] = apool.tile([1, QC], F32, tag="den1", name="den_f")
                nc.scalar.copy(den[1], ps_att[1][DK:VW, :])
                warm(2)
                for h in range(2):
                    rcp[h] = apool.tile([1, QC], F32, tag=f"denr{h}",
                                        name="den_rf")
                    nc.vector.reciprocal_approx_fast(rcp[h], den[h])
                    rr[h] = apool.tile([1, QC], mm_dt, tag=f"denrr{h}",
                                       name="den_rr")
                    nc.scalar.copy(rr[h], rcp[h])
                warm(2)
                # borrow a scores PSUM buffer (free after the last exp)
                bcp = spool.tile([DK, 2, QC], F32, tag="ss", name="bc_ps")
                for h in range(2):
                    nc.tensor.matmul(bcp[:, h, :], lhsT=ones_col_r,
                                     rhs=rr[h], start=True, stop=True)
                bc_sb = apool.tile([DK, 2, QC], F32, tag="bcs", bufs=1,
                                   name="bc_sb")
                nc.scalar.copy(bc_sb, bcp)
                warm(2)
                op = out_proj_thunks(n, att, qc,
                                     queues=[nc.gpsimd, nc.sync])
                for j in range(QC // LTS):
                    sl = slice(j * LTS, (j + 1) * LTS)
                    for h in range(2):
                        # normalize straight out of the PSUM accumulator
                        nc.vector.tensor_mul(att[DK * h:DK * (h + 1), sl],
                                             ps_att[h][0:DK, sl],
                                             bc_sb[:, h, sl])
                    op[2 * j]()
                    op[2 * j + 1]()
                    warm(1)

            # --- startup: weights + the three chunk-0 inputs split across
            # BOTH queues (parallel pull, projections start per-ck as
            # blocks land); chunk-1 K/V likewise split. Warm-up matmuls
            # above cover the DMA wait. ---
            nc.sync.dma_start(out=wq_s, in_=WQp)
            nc.gpsimd.dma_start(out=wv_s, in_=WVp)
            nc.gpsimd.dma_start(out=wo_s, in_=WOp)
            nc.sync.dma_start(out=wk_s, in_=WKp)
            # V chunk-0 rides the (at startup) idle ScalarE queue so
            # Q/K chunk-0 get both main queues to themselves.
            dma_issue8(QT, 0, 0, ("q", 0, 0), [nc.sync, nc.gpsimd])
            dma_issue8(KT, 0, 0, ("k", 0, 0), [nc.sync, nc.gpsimd])
            dma_issue8(VT, 0, 0, ("v", 0, 0), [nc.scalar])
            for key, src in ((("k", 0, 1), KT), (("v", 0, 1), VT)):
                for s in range(4):
                    cin = inp.tile([CK, 2, QC], mm_dt, tag="cin4", bufs=36,
                                   name="cin4")
                    q = nc.sync if s < 2 else nc.gpsimd
                    q.dma_start(out=cin,
                                in_=src[key[1], key[2], :, 2 * s:2 * s + 2, :])
                    stage[(key, 2 * s)] = cin[:, 0, :]
                    stage[(key, 2 * s + 1)] = cin[:, 1, :]

            # warm bursts between the DMA-gated projections keep every PE
            # idle window under the ~3.4us HAM re-throttle threshold
            for t in qk_mm(wq_s, seqs[0][0], 0, ("q", 0, 0)):
                t()
            warm(4)
            for t in qk_mm(wk_s, seqs[0][1], 0, ("k", 0, 0)):
                t()
            warm(4)
            for t in v_mm(seqs[0][2], 0, 0, ("v", 0, 0)):
                t()

            # deadline slots keyed by GLOBAL iteration git = ci*16 + lt
            kv01 = kv_proj(0, 1)
            kv02 = kv_proj(0, 2)
            kv03 = kv_proj(0, 3)
            kv13 = kv_proj(1, 3)
            slots = {
                0: dmas((KT, 0, 2, "k")),
                1: dmas_sp((VT, 0, 2, "v")),
                2: [kv01[0]],
                3: dmas((QT, 0, 1, "q")),
                4: [kv01[1]] + dmas_sp((KT, 0, 3, "k")),
                5: [kv01[2]],
                6: [kv02[0]] + dmas((VT, 0, 3, "v")),
                7: [kv02[1]],
                8: [kv02[2]] + dmas_sp((KT, 1, 0, "k")),
                9: dmas((VT, 1, 0, "v")),
                10: [kv03[0]],
                11: [kv03[1]],
                12: [kv03[2]] + dmas_sp((QT, 0, 2, "q")),
                13: qk_mm(wq_s, seqs[0][0], 1, ("q", 0, 1)),
                14: dmas((KT, 1, 1, "k")),
                # ci 1: kv(1,0), qproj(0,2)
                18: dmas_sp((VT, 1, 1, "v")),
                22: dmas((KT, 1, 2, "k")),
                26: dmas_sp((VT, 1, 2, "v")),
                28: qk_mm(wq_s, seqs[0][0], 2, ("q", 0, 2)),
                30: dmas((QT, 0, 3, "q")),
                # ci 2: kv(1,1), qproj(0,3)
                34: dmas_sp((KT, 1, 3, "k")),
                38: dmas((VT, 1, 3, "v")),
                42: dmas_sp((QT, 1, 0, "q")),
                44: qk_mm(wq_s, seqs[0][0], 3, ("q", 0, 3)),
                # ci 3: kv(1,2), qproj(1,0)
                50: dmas((QT, 1, 1, "q")),
                60: qk_mm(wq_s, seqs[1][0], 0, ("q", 1, 0)),
                # ci 4: kv(1,3) pinned early, qproj(1,1)
                65: [kv13[0]],
                66: [kv13[1]],
                67: [kv13[2]],
                68: dmas((QT, 1, 2, "q")),
                76: qk_mm(wq_s, seqs[1][0], 1, ("q", 1, 1)),
                84: dmas_sp((QT, 1, 3, "q")),
                92: qk_mm(wq_s, seqs[1][0], 2, ("q", 1, 2)),
                108: qk_mm(wq_s, seqs[1][0], 3, ("q", 1, 3)),
            }
            slots = {k: v for k, v in slots.items() if v}
            for g in range(4):      # bridge ci0's DMA-gated early iters
                slots.setdefault(g, []).append(lambda: warm(2))
            kv10 = kv_proj(1, 0)
            kv11 = kv_proj(1, 1)
            kv12 = kv_proj(1, 2)
            for base, kv in ((20, kv10), (36, kv11), (52, kv12)):
                for off, t in enumerate(kv):
                    slots.setdefault(base + 2 * off, []).append(t)

            # --- global iteration stream: scores/exp run one step ahead
            # of AV; norm + out-projection of each finished chunk drip
            # into the next chunk's iterations. No dense blocks, no
            # pipeline bubble at chunk boundaries. ---
            NCH = NB * NQC
            work = deque()
            ps_att = {}
            prev_e = None
            for git in range(NCH * NLT + 1):
                ci, lt = divmod(git, NLT)
                e = None
                if ci < NCH:
                    n, qc = divmod(ci, NQC)
                    if lt == 0:
                        ps_att[ci] = [
                            atpool.tile([VW, QC], F32, tag=f"pa{h}",
                                        name=f"ps_att{h}") for h in range(2)]
                    e = scores_pair(seqs[n][0], seqs[n][1], qc, lt)
                def do_av():
                    pci, plt = divmod(git - 1, NLT)
                    pn, pqc = divmod(pci, NQC)
                    av_pair(seqs[pn][2], ps_att[pci], prev_e, plt,
                            start=(plt == 0), stop=(plt == NLT - 1))
                    if plt == NLT - 1:
                        pa = ps_att.pop(pci)
                        if pci < NCH - 1:
                            # tA (inside norm_thunks) must hit the DVE
                            # queue before this git's drip casts: the next
                            # chunk's first AV reuses the accumulator.
                            att, pend, _ = norm_thunks(pa)
                            work.extend(pend)
                            work.extend(out_proj_thunks(pn, att, pqc))
                        else:
                            while work:
                                work.popleft()()
                            tail(pa, pn, pqc)
                if git >= 1 and lt == 0:
                    do_av()
                for t in slots.get(git, ()):
                    t()
                if work:
                    take = -(-len(work) // max(1, NLT - 1 - lt))
                    for _ in range(min(take, len(work))):
                        work.popleft()()
                if git >= 1 and lt != 0:
                    do_av()
                prev_e = e

        body()

    nc.compile()
    return nc


_CACHED = {}


def _get_program(key=("bf16",)):
    if key not in _CACHED:
        _CACHED[key] = build_program()
    return _CACHED[key]


def prep_inputs(Q, K, V, WQ, WK, WV, WO, bO):
    """Host-side shard prep: transposes + per-core weight slices."""
    import ml_dtypes
    wire = ml_dtypes.bfloat16
    Q = np.asarray(Q, dtype=np.float32)
    K = np.asarray(K, dtype=np.float32)
    V = np.asarray(V, dtype=np.float32)
    WQ = np.asarray(WQ, dtype=np.float32)
    WK = np.asarray(WK, dtype=np.float32)
    WV = np.asarray(WV, dtype=np.float32)
    WO = np.asarray(WO, dtype=np.float32)

    def blockT(X):
        # [N, T, D] -> X^T blocked [NB, NQC, CK, NCK, QC]; per (n, qc) the
        # [CK, NCK*QC] block is contiguous (one DMA, 8KB per partition)
        Xt = np.swapaxes(X, 1, 2).reshape(NB, NCK, CK, NQC, QC)
        return np.ascontiguousarray(
            Xt.transpose(0, 3, 2, 1, 4)).astype(wire)

    QT = blockT(Q)
    KT = blockT(K)
    VT = blockT(V)
    scale = 1.0 / math.sqrt(DK)

    def wblk(w):
        # [D, HP] -> [CK, NCK, HP] (d = k*CK + c -> [c, k, m]), contiguous
        return np.ascontiguousarray(
            w.reshape(NCK, CK, HP).transpose(1, 0, 2)).astype(wire)

    in_maps = []
    for p in range(N_CORES):
        sl = slice(HP * p, HP * (p + 1))
        in_maps.append({
            "QT": QT, "KT": KT, "VT": VT,
            "WQp": wblk(np.ascontiguousarray(WQ[:, sl]) * scale),
            "WKp": wblk(np.ascontiguousarray(WK[:, sl])),
            "WVp": wblk(np.ascontiguousarray(WV[:, sl])),
            "WOp": np.ascontiguousarray(WO[sl, :]).astype(wire),
        })
    return in_maps


def kernel(Q, K, V, WQ, WK, WV, WO, bO):
    nc = _get_program()
    in_maps = prep_inputs(Q, K, V, WQ, WK, WV, WO, bO)
    res = run_bass_kernel_spmd(nc, in_maps, list(range(N_CORES)))
    acc = np.zeros((NB, T, D), np.float32)
    for p in range(N_CORES):
        acc += res.results[p]["O"].astype(np.float32)
    return acc + np.asarray(bO, dtype=np.float32)


# revision 35
# speedup vs baseline: 1.2358x; 1.2358x over previous
"""Multi-head attention (N=2, T=2048, D=1024, H=16, dk=dv=64) on 8 TRN2 cores.

Sharding: tensor-parallel over heads. Core p computes heads {2p, 2p+1}
(a 128-wide slice of the QKV projections and of WO's rows), producing a
partial output [2, 2048, 1024]; the host sums the 8 partials and adds bO
(row-parallel linear => sum-reduce unshard).

Device algorithm (per core, per batch n):
  1. qT = (WQp/8).T @ Q.T   [128, 2048]   (scale 1/sqrt(dk) folded into WQp)
     kT = WKp.T @ K.T       [128, 2048]
     v  = V @ WVp           [128part(l-tile), 16, 2, dk+1] with a ones
          column appended per head (softmax denominator for free)
  2. scores in "KQ" orientation: S^T[l, q] = kT.T(l-tile) @ qT(q-chunk);
     the two heads run CONCURRENTLY on PE row groups 0-63 / 64-127.
  3. E = exp(S^T) on ScalarE (its only job), PSUM -> SBUF bf16.
  4. attnT_aug[dv+1, q] += v_aug.T(l-tile) @ E accumulated over l-tiles in
     PSUM; row dv holds sum(exp) = softmax denominator.
  5. normalize: att = attnT * bcast(1/den) (DVE; 1/den broadcast across
     partitions via a K=1 PE matmul)
  6. O^T-partial: out[q-tile, :] = att[:, q-tile].T @ WOp, written fp16.

Schedule: a warm-up burst of dummy matmuls un-throttles the PE HAM clock
gate (4/8 -> 8/8) during the initial DMA wait; after that every piece of
projection / normalize / out-projection work is dripped into the
exp-paced attention l-loops so the PE stream never idles (no HAM
re-throttle) and ScalarE runs an uninterrupted exp stream.  All
PSUM->SBUF casts are on DVE, all DMA issue on the SP/GpSimd queues.
"""

import math
import numpy as np
from contextlib import ExitStack
from collections import deque

import concourse.bass as bass
import concourse.tile as tile
from concourse import bacc, mybir
from concourse.bass_utils import run_bass_kernel_spmd

N_CORES = 8
NB, T, D = 2, 2048, 1024
HEADS, DK = 16, 64
HP = 2 * DK          # per-core head-pair width = 128
QC = 512             # query-chunk (matmul moving free dim)
NQC = T // QC        # 4
LTS = 128            # key/l tile (PE partition dim)
NLT = T // LTS       # 16
CK = 128             # contraction chunk for projections
NCK = D // CK        # 8
VW = DK + 1          # v columns per head incl. ones column

F32 = mybir.dt.float32
BF16 = mybir.dt.bfloat16
FP16 = mybir.dt.float16
EXP = mybir.ActivationFunctionType.Exp


def build_program(mm_dt=BF16, out_dt=FP16):
    """Build + compile the SPMD program (identical on all 8 cores)."""
    nc = bacc.Bacc("TRN2", target_bir_lowering=False, debug=False,
                   num_devices=N_CORES)
    # [NB, NQC, CK, NCK*QC]: per (n, chunk) a contiguous [128, 8KB] block
    # so each input chunk is ONE dma_start with 2KB+ descriptors.
    QT = nc.dram_tensor("QT", [NB, NQC, CK, NCK, QC], mm_dt,
                        kind="ExternalInput").ap()
    KT = nc.dram_tensor("KT", [NB, NQC, CK, NCK, QC], mm_dt,
                        kind="ExternalInput").ap()
    VT = nc.dram_tensor("VT", [NB, NQC, CK, NCK, QC], mm_dt,
                        kind="ExternalInput").ap()
    WQp = nc.dram_tensor("WQp", [CK, NCK, HP], mm_dt,
                         kind="ExternalInput").ap()
    WKp = nc.dram_tensor("WKp", [CK, NCK, HP], mm_dt,
                         kind="ExternalInput").ap()
    WVp = nc.dram_tensor("WVp", [CK, NCK, HP], mm_dt,
                         kind="ExternalInput").ap()
    WOp = nc.dram_tensor("WOp", [HP, D], mm_dt, kind="ExternalInput").ap()
    O = nc.dram_tensor("O", [NB, T, D], out_dt, kind="ExternalOutput").ap()

    with tile.TileContext(nc) as tc, ExitStack() as ctx:
        wpool = ctx.enter_context(tc.tile_pool(name="w", bufs=1))
        seq = ctx.enter_context(tc.tile_pool(name="seq", bufs=2))
        inp = ctx.enter_context(tc.tile_pool(name="inp", bufs=11))
        epool = ctx.enter_context(tc.tile_pool(name="e", bufs=8))
        apool = ctx.enter_context(tc.tile_pool(name="att", bufs=3))
        opool = ctx.enter_context(tc.tile_pool(name="o", bufs=6))
        ppool = ctx.enter_context(tc.tile_pool(name="pp", bufs=2, space="PSUM"))
        spool = ctx.enter_context(tc.tile_pool(name="ps", bufs=2, space="PSUM"))
        atpool = ctx.enter_context(tc.tile_pool(name="pa", bufs=1, space="PSUM"))

        # --- static SBUF: weights + constants ---
        wq_s = wpool.tile([CK, NCK, HP], mm_dt)
        wk_s = wpool.tile([CK, NCK, HP], mm_dt)
        wv_s = wpool.tile([CK, NCK, HP], mm_dt)
        wo_s = wpool.tile([HP, D], mm_dt)

        ones_col = wpool.tile([1, DK], F32, name="ones_col")
        nc.vector.memset(ones_col, 1.0)
        ones_col_r = wpool.tile([1, DK], mm_dt, name="ones_col_r")
        nc.vector.tensor_copy(ones_col_r, ones_col)
        # warm-up operand (zeros are fine for dummy matmuls)
        wsrc = wpool.tile([CK, QC], mm_dt, name="wsrc")
        nc.vector.memset(wsrc, 0.125)

        # --- HAM warm-up: dummy matmuls that run during the initial DMA
        # wait. ~8 N=512 matmuls at the cold 1.2GHz clock cover the 3.4us
        # busy window that flips the PE clock gate to 8/8; a few more keep
        # it busy until the first real projection's inputs land. ---
        for wi in range(12):
            pw = ppool.tile([HP, QC], F32, tag="pp", name="warm")
            nc.tensor.matmul(pw, lhsT=wsrc[:, 0:HP], rhs=wsrc,
                             start=True, stop=True)

        # --- staged input chunks (one tile + one DMA per (src, n, c)) ---
        stage = {}

        def dma_issue8(src, n, c, key, queues):
            """Startup chunks: 8 per-ck sub-DMAs spread across queues
            (each issue-stream only sustains ~45GB/s, so landing latency
            scales with stream count), per-ck tiles so each projection
            matmul starts as soon as its own 128KB block lands."""
            for ck in range(NCK):
                cin = inp.tile([CK, QC], mm_dt, tag="cin", bufs=24,
                               name="cin")
                queues[ck % len(queues)].dma_start(
                    out=cin, in_=src[n, c, :, ck, :])
                stage[(key, ck)] = cin

        def dma_in(src, n, c, key, eng=None):
            """Prefetch chunks: 4 quarter-chunk sub-DMAs — a balance of
            issue cost (~650ns per dma_start, the scarce mid-run resource)
            and landing latency (~45GB/s per issue-stream)."""
            def f():
                q = eng or nc.sync
                for s in range(4):
                    cin = inp.tile([CK, 2, QC], mm_dt, tag="cin4", bufs=36,
                                   name="cin4")
                    q.dma_start(out=cin, in_=src[n, c, :, 2 * s:2 * s + 2, :])
                    stage[(key, 2 * s)] = cin[:, 0, :]
                    stage[(key, 2 * s + 1)] = cin[:, 1, :]
            return f

        def qk_mm(w_s, dst, qc, key):
            """Fused projection: 8 accumulating matmuls + one DVE cast."""
            def f():
                ps = ppool.tile([HP, QC], F32, tag="pp", name="ps_proj")
                for ck in range(NCK):
                    nc.tensor.matmul(ps, lhsT=w_s[:, ck, :],
                                     rhs=stage.pop((key, ck)),
                                     start=(ck == 0), stop=(ck == NCK - 1))
                nc.vector.tensor_copy(dst[:, qc * QC:(qc + 1) * QC], ps)
            return [f]

        def v_mm(v_sb, n, c, key):
            """v natural-layout projection for token-chunk c (4 l-tiles);
            one [128, 2, 64] DVE copy per l-tile (ones cols untouched)."""
            cins = {}
            def grab():
                for ck in range(NCK):
                    cins[ck] = stage.pop((key, ck))
            def t(j):
                def f():
                    lt = c * (QC // LTS) + j
                    pv = ppool.tile([LTS, 2, DK], F32, tag="pp", name="pv")
                    for ck in range(NCK):
                        nc.tensor.matmul(
                            pv, lhsT=cins[ck][:, j * LTS:(j + 1) * LTS],
                            rhs=wv_s[:, ck, :],
                            start=(ck == 0), stop=(ck == NCK - 1))
                    nc.vector.tensor_copy(v_sb[:, lt, :, 0:DK], pv)
                return f
            return [grab] + [t(j) for j in range(QC // LTS)]

        def scores_pair(qT_sb, kT_sb, qc, lt):
            """S^T for both heads of (q-chunk, l-tile) into one 2-bank PSUM
            tile; heads run concurrently on PE row groups; single wide exp."""
            ss = spool.tile([LTS, 2 * QC], F32, tag="ss", name="ss")
            for h in range(2):
                nc.tensor.matmul(
                    ss[:, h * QC:(h + 1) * QC],
                    lhsT=kT_sb[DK * h:DK * (h + 1), lt * LTS:(lt + 1) * LTS],
                    rhs=qT_sb[DK * h:DK * (h + 1), qc * QC:(qc + 1) * QC],
                    start=True, stop=True)
            e = epool.tile([LTS, 2 * QC], mm_dt, tag="e", name="e")
            nc.scalar.activation(e, ss, EXP)
            return e

        def av_pair(v_sb, ps_att, e, lt, start, stop):
            for h in range(2):
                nc.tensor.matmul(ps_att[h],
                                 lhsT=v_sb[:, lt, h, :],
                                 rhs=e[:, h * QC:(h + 1) * QC],
                                 start=start, stop=stop)

        def norm_thunks(ps_att):
            """Softmax normalization for a finished accumulator pair. The
            PSUM quick-release copies (tA) are emitted INLINE so the next
            chunk's first AV matmul (which reuses the single-buffered
            accumulator) never convoys behind later DVE work. Returns
            (att, [C, D], tDj): C builds the 1/den broadcast, D applies."""
            att_raw = apool.tile([HP, QC], F32, tag="att_raw", name="att_raw")
            att = apool.tile([HP, QC], mm_dt, tag="attT", name="att")
            state = {}

            def tA(h):
                def f():
                    nc.vector.tensor_copy(att_raw[DK * h:DK * (h + 1), :],
                                          ps_att[h][0:DK, :])
                    den_f = apool.tile([1, QC], F32, tag=f"den{h}",
                                       name="den_f")
                    nc.vector.tensor_copy(den_f, ps_att[h][DK:VW, :])
                    state["den%d" % h] = den_f
                return f

            def tC():
                for h in range(2):
                    den_rf = apool.tile([1, QC], F32, tag=f"denr{h}",
                                        name="den_rf")
                    nc.vector.reciprocal_approx_fast(den_rf,
                                                     state["den%d" % h])
                    den_rr = apool.tile([1, QC], mm_dt, tag=f"denrr{h}",
                                        name="den_rr")
                    nc.vector.tensor_copy(den_rr, den_rf)
                    bcp = ppool.tile([DK, QC], F32, tag="pp", name="bc_ps")
                    nc.tensor.matmul(bcp, lhsT=ones_col_r, rhs=den_rr,
                                     start=True, stop=True)
                    state["bc%d" % h] = bcp

            def tD():
                for h in range(2):
                    nc.vector.tensor_mul(att[DK * h:DK * (h + 1), :],
                                         att_raw[DK * h:DK * (h + 1), :],
                                         state["bc%d" % h])

            def tDj(j):
                sl = slice(j * LTS, (j + 1) * LTS)
                for h in range(2):
                    nc.vector.tensor_mul(att[DK * h:DK * (h + 1), sl],
                                         att_raw[DK * h:DK * (h + 1), sl],
                                         state["bc%d" % h][:, sl])

            tA(0)()
            tA(1)()
            return att, [tC, tD], tDj

        def out_proj_thunks(n, att, qc, queues=None):
            """out-projection chunk: 8 (MM + fp16-cast) thunks, DMA per
            q-tile issued from `queues` (round-robin)."""
            queues = queues or [nc.gpsimd]
            box = {}
            thunks = []
            for j in range(QC // LTS):
                for half in range(2):
                    def t(j=j, half=half):
                        qt = qc * (QC // LTS) + j
                        if half == 0:
                            box[j] = opool.tile([LTS, D], out_dt, tag="osb",
                                                name="o_sb")
                        o_sb = box[j]
                        po = ppool.tile([LTS, QC], F32, tag="pp", name="po")
                        nc.tensor.matmul(
                            po, lhsT=att[:, j * LTS:(j + 1) * LTS],
                            rhs=wo_s[:, half * QC:(half + 1) * QC],
                            start=True, stop=True)
                        nc.vector.tensor_copy(
                            o_sb[:, half * QC:(half + 1) * QC], po)
                        if half == 1:
                            queues[j % len(queues)].dma_start(
                                out=O[n, qt * LTS:(qt + 1) * LTS, :],
                                in_=box.pop(j))
                    thunks.append(t)
            return thunks

        def body():
            seqs = []
            for n in range(NB):
                qT_sb = seq.tile([HP, T], mm_dt, tag="qT", name="qT_sb")
                kT_sb = seq.tile([HP, T], mm_dt, tag="kT", name="kT_sb")
                v_sb = seq.tile([LTS, NLT, 2, VW], mm_dt, tag="v",
                                name="v_sb")
                nc.vector.memset(v_sb[:, :, :, DK], 1.0)
                seqs.append((qT_sb, kT_sb, v_sb))

            def kv_proj(n, c):
                """3 drip thunks: k-proj burst, then v j-tiles 0-1, 2-3."""
                k = qk_mm(wk_s, seqs[n][1], c, ("k", n, c))
                v = v_mm(seqs[n][2], n, c, ("v", n, c))
                return [k[0], lambda: [t() for t in v[0:3]],
                        lambda: [t() for t in v[3:5]]]

            def dmas(*specs):
                """One thunk per chunk DMA (4 issues each) on gpsimd."""
                return [dma_in(src, n, c, (pfx, n, c), eng=nc.gpsimd)
                        for (src, n, c, pfx) in specs]

            def dmas_sp(*specs):
                """Same, issued from the SP queue."""
                return [dma_in(src, n, c, (pfx, n, c))
                        for (src, n, c, pfx) in specs]

            def warm(k):
                for _ in range(k):
                    pw = ppool.tile([HP, QC], F32, tag="pp", name="warm")
                    nc.tensor.matmul(pw, lhsT=wsrc[:, 0:HP], rhs=wsrc,
                                     start=True, stop=True)

            def tail(ps_att, n, qc):
                """Final-chunk normalize + out-project, pipelined per
                q-tile with the norm chain split across DVE and the (now
                idle) ScalarE; warm matmuls keep the HAM clock at 8/8."""
                att = apool.tile([HP, QC], mm_dt, tag="attT", name="att")
                den, rcp, rr = {}, {}, {}
                den[0] = apool.tile([1, QC], F32, tag="den0", name="den_f")
                nc.vector.tensor_copy(den[0], ps_att[0][DK:VW, :])
                den[1] = apool.tile([1, QC], F32, tag="den1", name="den_f")
                nc.scalar.copy(den[1], ps_att[1][DK:VW, :])
                warm(2)
                for h in range(2):
                    rcp[h] = apool.tile([1, QC], F32, tag=f"denr{h}",
                                        name="den_rf")
                    nc.vector.reciprocal_approx_fast(rcp[h], den[h])
                    rr[h] = apool.tile([1, QC], mm_dt, tag=f"denrr{h}",
                                       name="den_rr")
                    nc.scalar.copy(rr[h], rcp[h])
                warm(2)
                # borrow a scores PSUM buffer (free after the last exp)
                bcp = spool.tile([DK, 2, QC], F32, tag="ss", name="bc_ps")
                for h in range(2):
                    nc.tensor.matmul(bcp[:, h, :], lhsT=ones_col_r,
                                     rhs=rr[h], start=True, stop=True)
                bc_sb = apool.tile([DK, 2, QC], F32, tag="bcs", bufs=1,
                                   name="bc_sb")
                nc.scalar.copy(bc_sb, bcp)
                warm(2)
                op = out_proj_thunks(n, att, qc,
                                     queues=[nc.gpsimd, nc.sync])
                for j in range(QC // LTS):
                    sl = slice(j * LTS, (j + 1) * LTS)
                    for h in range(2):
                        # normalize straight out of the PSUM accumulator
                        nc.vector.tensor_mul(att[DK * h:DK * (h + 1), sl],
                                             ps_att[h][0:DK, sl],
                                             bc_sb[:, h, sl])
                    op[2 * j]()
                    op[2 * j + 1]()
                    warm(1)

            # --- startup: weights + the three chunk-0 inputs split across
            # BOTH queues (parallel pull, projections start per-ck as
            # blocks land); chunk-1 K/V likewise split. Warm-up matmuls
            # above cover the DMA wait. ---
            nc.sync.dma_start(out=wq_s, in_=WQp)
            nc.gpsimd.dma_start(out=wv_s, in_=WVp)
            nc.gpsimd.dma_start(out=wo_s, in_=WOp)
            nc.sync.dma_start(out=wk_s, in_=WKp)
            # NOTE: SP and ScalarE share the one hardware DGE; GpSimd has
            # the software DGE — only two independent issue paths exist.
            dma_issue8(QT, 0, 0, ("q", 0, 0), [nc.sync, nc.gpsimd])
            dma_issue8(KT, 0, 0, ("k", 0, 0), [nc.sync, nc.gpsimd])
            dma_issue8(VT, 0, 0, ("v", 0, 0), [nc.sync, nc.gpsimd])
            for key, src in ((("k", 0, 1), KT), (("v", 0, 1), VT)):
                for s in range(4):
                    cin = inp.tile([CK, 2, QC], mm_dt, tag="cin4", bufs=36,
                                   name="cin4")
                    q = nc.sync if s < 2 else nc.gpsimd
                    q.dma_start(out=cin,
                                in_=src[key[1], key[2], :, 2 * s:2 * s + 2, :])
                    stage[(key, 2 * s)] = cin[:, 0, :]
                    stage[(key, 2 * s + 1)] = cin[:, 1, :]

            # warm bursts between the DMA-gated projections keep every PE
            # idle window under the ~3.4us HAM re-throttle threshold
            for t in qk_mm(wq_s, seqs[0][0], 0, ("q", 0, 0)):
                t()
            warm(4)
            for t in qk_mm(wk_s, seqs[0][1], 0, ("k", 0, 0)):
                t()
            warm(4)
            for t in v_mm(seqs[0][2], 0, 0, ("v", 0, 0)):
                t()

            # deadline slots keyed by GLOBAL iteration git = ci*16 + lt
            kv01 = kv_proj(0, 1)
            kv02 = kv_proj(0, 2)
            kv03 = kv_proj(0, 3)
            kv13 = kv_proj(1, 3)
            slots = {
                0: dmas((KT, 0, 2, "k")),
                1: dmas_sp((VT, 0, 2, "v")),
                2: [kv01[0]],
                3: dmas((QT, 0, 1, "q")),
                4: [kv01[1]] + dmas_sp((KT, 0, 3, "k")),
                5: [kv01[2]],
                6: [kv02[0]] + dmas((VT, 0, 3, "v")),
                7: [kv02[1]],
                8: [kv02[2]] + dmas_sp((KT, 1, 0, "k")),
                9: dmas((VT, 1, 0, "v")),
                10: [kv03[0]],
                11: [kv03[1]],
                12: [kv03[2]] + dmas_sp((QT, 0, 2, "q")),
                13: qk_mm(wq_s, seqs[0][0], 1, ("q", 0, 1)),
                14: dmas((KT, 1, 1, "k")),
                # ci 1: kv(1,0), qproj(0,2)
                18: dmas_sp((VT, 1, 1, "v")),
                22: dmas((KT, 1, 2, "k")),
                26: dmas_sp((VT, 1, 2, "v")),
                28: qk_mm(wq_s, seqs[0][0], 2, ("q", 0, 2)),
                30: dmas((QT, 0, 3, "q")),
                # ci 2: kv(1,1), qproj(0,3)
                34: dmas_sp((KT, 1, 3, "k")),
                38: dmas((VT, 1, 3, "v")),
                42: dmas_sp((QT, 1, 0, "q")),
                44: qk_mm(wq_s, seqs[0][0], 3, ("q", 0, 3)),
                # ci 3: kv(1,2), qproj(1,0)
                50: dmas((QT, 1, 1, "q")),
                60: qk_mm(wq_s, seqs[1][0], 0, ("q", 1, 0)),
                # ci 4: kv(1,3) pinned early, qproj(1,1)
                65: [kv13[0]],
                66: [kv13[1]],
                67: [kv13[2]],
                68: dmas((QT, 1, 2, "q")),
                76: qk_mm(wq_s, seqs[1][0], 1, ("q", 1, 1)),
                84: dmas_sp((QT, 1, 3, "q")),
                92: qk_mm(wq_s, seqs[1][0], 2, ("q", 1, 2)),
                108: qk_mm(wq_s, seqs[1][0], 3, ("q", 1, 3)),
            }
            slots = {k: v for k, v in slots.items() if v}
            for g in range(4):      # bridge ci0's DMA-gated early iters
                slots.setdefault(g, []).append(lambda: warm(2))
            kv10 = kv_proj(1, 0)
            kv11 = kv_proj(1, 1)
            kv12 = kv_proj(1, 2)
            for base, kv in ((20, kv10), (36, kv11), (52, kv12)):
                for off, t in enumerate(kv):
                    slots.setdefault(base + 2 * off, []).append(t)

            # --- global iteration stream: scores/exp run one step ahead
            # of AV; norm + out-projection of each finished chunk drip
            # into the next chunk's iterations. No dense blocks, no
            # pipeline bubble at chunk boundaries. ---
            NCH = NB * NQC
            work = deque()
            ps_att = {}
            prev_e = None
            for git in range(NCH * NLT + 1):
                ci, lt = divmod(git, NLT)
                e = None
                if ci < NCH:
                    n, qc = divmod(ci, NQC)
                    if lt == 0:
                        ps_att[ci] = [
                            atpool.tile([VW, QC], F32, tag=f"pa{h}",
                                        name=f"ps_att{h}") for h in range(2)]
                    e = scores_pair(seqs[n][0], seqs[n][1], qc, lt)
                def do_av():
                    pci, plt = divmod(git - 1, NLT)
                    pn, pqc = divmod(pci, NQC)
                    av_pair(seqs[pn][2], ps_att[pci], prev_e, plt,
                            start=(plt == 0), stop=(plt == NLT - 1))
                    if plt == NLT - 1:
                        pa = ps_att.pop(pci)
                        if pci < NCH - 1:
                            # tA (inside norm_thunks) must hit the DVE
                            # queue before this git's drip casts: the next
                            # chunk's first AV reuses the accumulator.
                            att, pend, _ = norm_thunks(pa)
                            work.extend(pend)
                            work.extend(out_proj_thunks(pn, att, pqc))
                        else:
                            while work:
                                work.popleft()()
                            tail(pa, pn, pqc)
                if git >= 1 and lt == 0:
                    do_av()
                for t in slots.get(git, ()):
                    t()
                if work:
                    take = -(-len(work) // max(1, NLT - 1 - lt))
                    for _ in range(min(take, len(work))):
                        work.popleft()()
                if git >= 1 and lt != 0:
                    do_av()
                prev_e = e

        body()

    nc.compile()
    return nc


_CACHED = {}


def _get_program(key=("bf16",)):
    if key not in _CACHED:
        _CACHED[key] = build_program()
    return _CACHED[key]


def prep_inputs(Q, K, V, WQ, WK, WV, WO, bO):
    """Host-side shard prep: transposes + per-core weight slices."""
    import ml_dtypes
    wire = ml_dtypes.bfloat16
    Q = np.asarray(Q, dtype=np.float32)
    K = np.asarray(K, dtype=np.float32)
    V = np.asarray(V, dtype=np.float32)
    WQ = np.asarray(WQ, dtype=np.float32)
    WK = np.asarray(WK, dtype=np.float32)
    WV = np.asarray(WV, dtype=np.float32)
    WO = np.asarray(WO, dtype=np.float32)

    def blockT(X):
        # [N, T, D] -> X^T blocked [NB, NQC, CK, NCK, QC]; per (n, qc) the
        # [CK, NCK*QC] block is contiguous (one DMA, 8KB per partition)
        Xt = np.swapaxes(X, 1, 2).reshape(NB, NCK, CK, NQC, QC)
        return np.ascontiguousarray(
            Xt.transpose(0, 3, 2, 1, 4)).astype(wire)

    QT = blockT(Q)
    KT = blockT(K)
    VT = blockT(V)
    scale = 1.0 / math.sqrt(DK)

    def wblk(w):
        # [D, HP] -> [CK, NCK, HP] (d = k*CK + c -> [c, k, m]), contiguous
        return np.ascontiguousarray(
            w.reshape(NCK, CK, HP).transpose(1, 0, 2)).astype(wire)

    in_maps = []
    for p in range(N_CORES):
        sl = slice(HP * p, HP * (p + 1))
        in_maps.append({
            "QT": QT, "KT": KT, "VT": VT,
            "WQp": wblk(np.ascontiguousarray(WQ[:, sl]) * scale),
            "WKp": wblk(np.ascontiguousarray(WK[:, sl])),
            "WVp": wblk(np.ascontiguousarray(WV[:, sl])),
            "WOp": np.ascontiguousarray(WO[sl, :]).astype(wire),
        })
    return in_maps


def kernel(Q, K, V, WQ, WK, WV, WO, bO):
    nc = _get_program()
    in_maps = prep_inputs(Q, K, V, WQ, WK, WV, WO, bO)
    res = run_bass_kernel_spmd(nc, in_maps, list(range(N_CORES)))
    acc = np.zeros((NB, T, D), np.float32)
    for p in range(N_CORES):
        acc += res.results[p]["O"].astype(np.float32)
    return acc + np.asarray(bO, dtype=np.float32)
